# revision 6
# baseline (speedup 1.0000x reference)
"""Trainium2 Bass kernel for the bipartite GNN recommender (8 NeuronCores).

Redesigned layout (v2):
- Node j -> core j%8. Per-core user rows l=j//8 in [0,25088), products
  l=25088+(p//8) in [25088,37760). Graph edges only touch nodes <200000
  (the reference never offsets prod_idx), so products are self-loop-only
  and their whole chain (proj->conv1->conv2->ts) is computed locally in
  phase P1 with zero collective traffic.
- Conv tables are block-laid per (half, core): user table row for node j:
  l<12544 -> c*12544+l, else 100352+c*12544+(l-12544). One AllGather per
  half, triggered as soon as that half's tiles are produced (overlaps
  compute). Products in ts table at 200704+c*12672+(p//8).
- Scatter segment-sum via transposed one-hot matmuls: stationary = the
  64-col message tile (half the LDWEIGHTS cost), stream = the one-hot,
  accumulate [64,128] per tile in one [64,512] PSUM bank per group.
- Self-term from a persistent SBUF copy of the core's own table slice
  (no indirect gather), dis applied via a persistent [64,25088] bf16
  broadcast table, biases via per-partition activation bias columns.
- Final pair-MLP: |W2| folded into the ts tables (pos/neg dim split on
  host), so per edge: gather t,s -> add -> relu -> two strided reduces
  -> subtract -> sigmoid(scale)*5.
"""
import ml_dtypes
import numpy as np

from concourse import bass, mybir, tile
from concourse.bass import AP, IndirectOffsetOnAxis
from concourse.bass_utils import run_bass_kernel_spmd
from concourse.masks import make_identity
from concourse.tile import add_dep_helper

F32 = mybir.dt.float32
BF16 = mybir.dt.bfloat16
F8 = mybir.dt.float8e4
I32 = mybir.dt.int32

AF = mybir.ActivationFunctionType
ALU = mybir.AluOpType

N_CORES = 8
NU, NP, NE = 200000, 100000, 1000000
SHARD = 37760
PU = 25088            # user rows per core
PC = 12672            # product rows per core
P0 = 12544            # rows per user half per core
UH2 = 8 * P0          # 100352
PROD_BASE = 2 * UH2   # 200704
TAB = PROD_BASE + 8 * PC  # 302080
TILES_A, TILES_B, TILES_C = 98, 98, 99
EPT = NE // N_CORES
NCH = 984             # pred output cols (125000 edges -> 977, pad to mult of 8)


# --------------------------------------------------------------------------
# legalization: this walrus build allows at most 1 sync wait per instruction
# --------------------------------------------------------------------------
def _split_sync_waits(nc, max_waits=1):
    import bass_rust
    for bb in nc.main_func.blocks:
        out = []
        for inst in bb.instructions:
            si = inst.sync_info
            if si is not None and si.on_wait is not None and len(si.on_wait) > max_waits:
                waits = list(si.on_wait)
                keep, extra = waits[-max_waits:], waits[:-max_waits]
                while extra:
                    chunk, extra = extra[:max_waits], extra[max_waits:]
                    nop = bass_rust.InstNoOp(name=f"I-{nc.next_id()}", ins=[], outs=[])
                    nop.engine = inst.engine
                    nop.bass_nofuse = True
                    nop.sync_info = mybir.SyncInfo(on_wait=chunk, on_update=[])
                    nc.register_instruction(nop, overwrite=True)
                    out.append(nop)
                si.on_wait = keep
            out.append(inst)
        del bb.instructions[:]
        for i in out:
            bb.add_instruction(i)


# --------------------------------------------------------------------------
# host-side sharding / layout prep
# --------------------------------------------------------------------------
def _pi_user(j):
    j = np.asarray(j, np.int64)
    c, l = j % 8, j // 8
    return np.where(l < P0, c * P0 + l, UH2 + c * P0 + (l - P0)).astype(np.int32)


def _pi_prod(p):
    p = np.asarray(p, np.int64)
    return (PROD_BASE + (p % 8) * PC + p // 8).astype(np.int32)


def _tile_blocks(l_sorted, vals_rows, n_tiles, K, t0):
    """Edges sorted by local dest row -> per-tile padded (rows, cols)."""
    t = (l_sorted >> 7) - t0
    start = np.searchsorted(t, np.arange(n_tiles))
    pos = np.arange(len(t)) - start[t]
    assert len(t) == 0 or pos.max() < K, "tile overflow"
    rows = np.zeros((n_tiles, 128, K // 128), np.int32)
    cols = np.full((n_tiles, 128, K // 128), -1.0, np.float32)
    rows[t, pos % 128, pos // 128] = vals_rows
    cols[t, pos % 128, pos // 128] = (l_sorted & 127).astype(np.float32)
    return rows, cols


def _groupify(arr3, nch):
    """[tiles, 128, nch] -> [ngroups*128, 4*nch] (4 tiles side by side)."""
    tiles = arr3.shape[0]
    ng = (tiles + 3) // 4
    out = np.zeros((ng * 128, 4 * nch), arr3.dtype)
    if arr3.dtype != np.int32:
        out[:] = -1.0
    for g in range(ng):
        for t in range(min(4, tiles - 4 * g)):
            out[g * 128:(g + 1) * 128, t * nch:(t + 1) * nch] = arr3[4 * g + t]
    return out


def _prepare(inputs):
    ei = np.asarray(inputs["edge_index"])
    u_idx = ei[0].astype(np.int64)
    p_idx = ei[1].astype(np.int64)

    # directed messages: src -> dst; both endpoints are node ids < 200000
    src = np.concatenate([u_idx, p_idx])
    dst = np.concatenate([p_idx, u_idx])
    core = (dst % 8).astype(np.int64)
    l = (dst // 8).astype(np.int64)
    src_pi = _pi_user(src)

    order = np.argsort(core * (1 << 32) + l, kind="stable")
    core_s, l_s, srcpi_s = core[order], l[order], src_pi[order]
    core_starts = np.searchsorted(core_s, np.arange(N_CORES + 1))

    cnt = np.bincount(core_s * 256 + (l_s >> 7), minlength=N_CORES * 256).reshape(N_CORES, 256)
    K_A = max(128, int(np.ceil(cnt[:, :TILES_A].max() / 128)) * 128)
    K_B = max(128, int(np.ceil(cnt[:, TILES_A:TILES_A + TILES_B].max() / 128)) * 128)
    nchA, nchB = K_A // 128, K_B // 128

    fw = np.asarray(inputs["user_features"], np.float32)
    pw = np.asarray(inputs["product_features"], np.float32)
    ue = np.asarray(inputs["user_emb"], np.float32)
    pe = np.asarray(inputs["product_emb"], np.float32)
    b_uf = np.asarray(inputs["b_uf"], np.float32)
    b_pf = np.asarray(inputs["b_pf"], np.float32)

    pi_u = _pi_user(u_idx)
    pi_p = _pi_prod(p_idx)

    # pred-MLP folding: permute hidden dims so W2>=0 dims come first,
    # scale W1 columns (and pb1) by |W2|*G, recover with sigmoid scale 1/G.
    W1 = np.asarray(inputs["pred_W1"], np.float32)     # [128, 64]
    w2 = np.asarray(inputs["pred_W2"], np.float32).reshape(64)
    pb1 = np.asarray(inputs["pred_b1"], np.float32)
    perm = np.argsort(w2 < 0, kind="stable")           # positives first
    npos = int((w2 >= 0).sum())
    aw = np.abs(w2[perm])
    amax = max(aw.max(), 1e-30)
    G = 1.0 / amax
    colscale = aw * G                                  # in (0, 1]
    W1s = W1[:, perm] * colscale[None, :]
    pb1s = pb1[perm] * colscale
    inv_g = float(amax)                                # sigmoid scale

    per_core = []
    for c in range(N_CORES):
        s0, s1 = core_starts[c], core_starts[c + 1]
        lc, sc = l_s[s0:s1], srcpi_s[s0:s1]
        mA = lc < TILES_A * 128
        rowsA, colsA = _tile_blocks(lc[mA], sc[mA], TILES_A, K_A, 0)
        rowsB, colsB = _tile_blocks(lc[~mA], sc[~mA], TILES_B, K_B, TILES_A)
        rows4A = _groupify(rowsA, nchA)
        cols4A = _groupify(colsA, nchA).astype(ml_dtypes.bfloat16)
        rows4B = _groupify(rowsB, nchB)
        cols4B = _groupify(colsB, nchB).astype(ml_dtypes.bfloat16)

        featT = np.zeros((128, SHARD), np.float32)
        embT = np.zeros((64, SHARD), np.float32)
        featT[:, :25000] = fw[c::8].T
        featT[:, 25088:37588] = pw[c::8].T
        embT[:, :25000] = ue[c::8].T + b_uf[:, None]
        embT[:, 25088:37588] = pe[c::8].T + b_pf[:, None]
        embT[:, 25000:25088] = b_uf[:, None]
        embT[:, 37588:] = b_pf[:, None]

        deg = np.bincount(lc, minlength=PU).astype(np.float32)
        dis = 1.0 / np.sqrt(deg + 1.0)
        disTu = np.tile(dis[None, :], (64, 1)).astype(ml_dtypes.bfloat16)

        e0 = c * EPT
        offU = np.zeros((128, NCH), np.int32)
        offP = np.zeros((128, NCH), np.int32)
        el = np.arange(EPT)
        offU[el % 128, el // 128] = pi_u[e0:e0 + EPT]
        offP[el % 128, el // 128] = pi_p[e0:e0 + EPT]
        offUP = np.zeros((128, 2 * NCH), np.int32)
        for g in range(NCH // 8):
            offUP[:, 16 * g:16 * g + 8] = offU[:, 8 * g:8 * g + 8]
            offUP[:, 16 * g + 8:16 * g + 16] = offP[:, 8 * g:8 * g + 8]

        per_core.append(dict(
            featT=featT.astype(ml_dtypes.bfloat16), embT=embT.astype(ml_dtypes.bfloat16),
            disTu=disTu, rows4A=rows4A, cols4A=cols4A, rows4B=rows4B, cols4B=cols4B,
            offUP=offUP,
        ))

    shared = dict(
        Wuf=np.asarray(inputs["W_uf"], np.float32).astype(ml_dtypes.bfloat16),
        Wpf=np.asarray(inputs["W_pf"], np.float32).astype(ml_dtypes.bfloat16),
        W1c=np.asarray(inputs["conv1_W"], np.float32).astype(ml_dtypes.bfloat16),
        W2c=np.asarray(inputs["conv2_W"], np.float32).astype(ml_dtypes.bfloat16),
        pW1t=np.ascontiguousarray(W1s[:64]).astype(ml_dtypes.bfloat16),
        pW1b=np.ascontiguousarray(W1s[64:]).astype(ml_dtypes.bfloat16),
        b1col=np.asarray(inputs["conv1_b"], np.float32).reshape(64, 1),
        b2col=np.asarray(inputs["conv2_b"], np.float32).reshape(64, 1),
        pb1col=pb1s.reshape(64, 1).astype(np.float32),
        b2pred=np.full((128, 1), float(np.asarray(inputs["pred_b2"]).reshape(())), np.float32),
    )
    return per_core, shared, nchA, nchB, npos, inv_g


# --------------------------------------------------------------------------
# numpy simulator of the device program (for host-side validation only)
# --------------------------------------------------------------------------
def _simulate(inputs):
    f8 = lambda x: np.asarray(x, np.float32).astype(ml_dtypes.float8_e4m3).astype(np.float32)
    bf = lambda x: np.asarray(x, np.float32).astype(ml_dtypes.bfloat16).astype(np.float32)
    per_core, shared, nchA, nchB, npos, inv_g = _prepare(inputs)
    Wuf, Wpf = bf(shared["Wuf"]), bf(shared["Wpf"])
    W1c, W2c = bf(shared["W1c"]), bf(shared["W2c"])
    pW1t, pW1b = bf(shared["pW1t"]), bf(shared["pW1b"])
    b1, b2 = shared["b1col"][:, 0], shared["b2col"][:, 0]
    pb1 = shared["pb1col"][:, 0]

    y1_t = np.zeros((PROD_BASE, 64), np.float32)
    ts_t = np.zeros((TAB, 64), np.float32)
    y1ownT, disT, featsT, embsT = [], [], [], []
    for c in range(N_CORES):
        pc = per_core[c]
        ft, et = bf(pc["featT"]), bf(pc["embT"])
        dis = bf(pc["disTu"])[0]  # [PU]
        x0 = bf(ft.T @ Wuf + et.T)              # [SHARD, 64] (user cols valid)
        y1 = f8((x0[:PU] @ W1c) * dis[:, None])
        y1ownT.append(y1)
        disT.append(dis)
        featsT.append(ft)
        embsT.append(et)
        for h, base in ((0, 0), (1, UH2)):
            y1_t[base + c * P0: base + (c + 1) * P0] = y1[h * P0:(h + 1) * P0]
        # region C local chain
        x0c = bf(ft[:, PU:].T @ Wpf + et[:, PU:].T)
        x1c = np.maximum(bf(x0c @ W1c) + b1, 0.0)
        x2c = bf(bf(x1c) @ W2c) + b2
        tsc = f8(bf(x2c) @ pW1b)
        ts_t[PROD_BASE + c * PC: PROD_BASE + (c + 1) * PC] = tsc

    def conv(y_t, layer):
        y2_t = np.zeros((PROD_BASE, 64), np.float32)
        outs = []
        for c in range(N_CORES):
            pc = per_core[c]
            dis = disT[c]
            aggT = np.zeros((64, PU), np.float32)
            for reg, (tiles, nch, r4, c4, t0) in enumerate((
                    (TILES_A, nchA, pc["rows4A"], pc["cols4A"], 0),
                    (TILES_B, nchB, pc["rows4B"], pc["cols4B"], TILES_A))):
                ng = (tiles + 3) // 4
                for g in range(ng):
                    gt = min(4, tiles - 4 * g)
                    rw = r4[g * 128:(g + 1) * 128]
                    cl = np.asarray(c4[g * 128:(g + 1) * 128], np.float32)
                    msg = f8(y_t[rw])           # [128, 4nch, 64]
                    for t in range(gt):
                        acc = np.zeros((64, 128), np.float32)
                        for j in range(nch):
                            ch = t * nch + j
                            S = (cl[:, ch:ch + 1] == np.arange(128)[None, :]).astype(np.float32)
                            acc += msg[:, ch].T @ S
                        tile_i = t0 + 4 * g + t
                        aggT[:, tile_i * 128:(tile_i + 1) * 128] = acc
            own = y1ownT[c] if layer == 1 else yown2[c]
            agg = aggT.T + own
            x = bf(agg * dis[:, None])
            if layer == 1:
                x1 = bf(np.maximum(x + b1, 0.0))
                y2 = f8(bf(x1 @ W2c) * dis[:, None])
                outs.append(y2)
                for h, base in ((0, 0), (1, UH2)):
                    y2_t[base + c * P0: base + (c + 1) * P0] = y2[h * P0:(h + 1) * P0]
            else:
                x2 = bf(x + b2)
                ts = f8(bf(x2 @ pW1t) + pb1)
                outs.append(ts)
                for h, base in ((0, 0), (1, UH2)):
                    ts_t[base + c * P0: base + (c + 1) * P0] = ts[h * P0:(h + 1) * P0]
        return y2_t, outs

    yown2 = None
    y2_t, yown2 = conv(y1_t, 1)
    _, _ = conv(y2_t, 2)

    # P7
    out = np.zeros(NE, np.float32)
    ei = np.asarray(inputs["edge_index"])
    pi_u = _pi_user(ei[0].astype(np.int64))
    pi_p = _pi_prod(ei[1].astype(np.int64))
    t = ts_t[pi_u]
    s = ts_t[pi_p]
    h = bf(np.maximum(bf(t + s), 0.0))
    logit = h[:, :npos].sum(1) - h[:, npos:].sum(1)
    z = logit * inv_g + float(np.asarray(inputs["pred_b2"]).reshape(()))
    out[:] = 5.0 / (1.0 + np.exp(-z))
    return out


# --------------------------------------------------------------------------
# device program
# --------------------------------------------------------------------------
def _v3(ap, mid, inner, mid_stride=None, inner_stride=0):
    a = ap.ap
    ms = a[1][0] if mid_stride is None else mid_stride
    return AP(ap.tensor, ap.offset, [list(a[0]), [ms, mid], [inner_stride, inner]])


def _o3(ap, nsub):
    return AP(ap.tensor, ap.offset, [list(ap.ap[0]), [128, nsub], [1, 128]])


def build_program(nchA, nchB, npos, inv_g):
    nc = bass.Bass("TRN2", target_bir_lowering=False, debug=False, num_devices=N_CORES)

    dp = nc.declare_dram_parameter
    featT_d = dp("featT", [128, SHARD], BF16, isOutput=False)
    embT_d = dp("embT", [64, SHARD], BF16, isOutput=False)
    disTu_d = dp("disTu", [64, PU], BF16, isOutput=False)
    rows4A_d = dp("rows4A", [25 * 128, 4 * nchA], I32, isOutput=False)
    cols4A_d = dp("cols4A", [25 * 128, 4 * nchA], BF16, isOutput=False)
    rows4B_d = dp("rows4B", [25 * 128, 4 * nchB], I32, isOutput=False)
    cols4B_d = dp("cols4B", [25 * 128, 4 * nchB], BF16, isOutput=False)
    offUP_d = dp("offUP", [128, 2 * NCH], I32, isOutput=False)
    Wuf_d = dp("Wuf", [128, 64], BF16, isOutput=False)
    Wpf_d = dp("Wpf", [128, 64], BF16, isOutput=False)
    W1c_d = dp("W1c", [64, 64], BF16, isOutput=False)
    W2c_d = dp("W2c", [64, 64], BF16, isOutput=False)
    pW1t_d = dp("pW1t", [64, 64], BF16, isOutput=False)
    pW1b_d = dp("pW1b", [64, 64], BF16, isOutput=False)
    b1col_d = dp("b1col", [64, 1], F32, isOutput=False)
    b2col_d = dp("b2col", [64, 1], F32, isOutput=False)
    pb1col_d = dp("pb1col", [64, 1], F32, isOutput=False)
    b2pred_d = dp("b2pred", [128, 1], F32, isOutput=False)
    preds_d = dp("preds", [128, NCH], F32, isOutput=True)

    with tile.TileContext(nc) as tc:
        with tc.tile_pool(name="const", bufs=1) as cp, \
             tc.tile_pool(name="sb", bufs=3) as sb, \
             tc.tile_pool(name="ps", bufs=2, space="PSUM") as ps, \
             tc.tile_pool(name="pssc", bufs=2, space="PSUM") as pssc, \
             tc.tile_pool(name="pst", bufs=2, space="PSUM") as pst:

            def reg_dge(h):
                mloc = nc.lookup_mloc(h)
                if mloc.table_entry_id is None:
                    mloc.table_entry_id = len(nc.dge_table) + 1
                    nc.dge_table.append(mloc.name)
                return h

            ag1_in = reg_dge(nc.dram_tensor("ag1_in", [PU, 64], F8))
            ag2_in = reg_dge(nc.dram_tensor("ag2_in", [PU, 64], F8))
            ag3_in = reg_dge(nc.dram_tensor("ag3_in", [SHARD, 64], F8))
            y1_t = reg_dge(nc.dram_tensor("y1_t", [PROD_BASE, 64], F8, addr_space="Shared"))
            y2_t = reg_dge(nc.dram_tensor("y2_t", [PROD_BASE, 64], F8, addr_space="Shared"))
            ts_t = reg_dge(nc.dram_tensor("ts_t", [TAB, 64], F8, addr_space="Shared"))

            # ---- constants ----
            iota_i = cp.tile([128, 128], I32, tag="iota_i")
            nc.gpsimd.iota(iota_i[:], [[1, 128]], channel_multiplier=0)
            iota_b = cp.tile([128, 128], BF16, tag="iota_b")
            nc.vector.tensor_copy(out=iota_b[:], in_=iota_i[:])
            idn = cp.tile([128, 128], F32, tag="idn")
            make_identity(nc, idn[:])
            idn8 = cp.tile([128, 128], F8, tag="idn8")
            nc.vector.tensor_copy(out=idn8[:], in_=idn[:])

            Wuf = cp.tile([128, 64], BF16, tag="Wuf")
            nc.sync.dma_start(out=Wuf[:], in_=Wuf_d[:])
            Wpf = cp.tile([128, 64], BF16, tag="Wpf")
            nc.sync.dma_start(out=Wpf[:], in_=Wpf_d[:])
            W1c = cp.tile([64, 64], BF16, tag="W1c")
            nc.sync.dma_start(out=W1c[:], in_=W1c_d[:])
            W2c = cp.tile([64, 64], BF16, tag="W2c")
            nc.sync.dma_start(out=W2c[:], in_=W2c_d[:])
            pW1t = cp.tile([64, 64], BF16, tag="pW1t")
            nc.sync.dma_start(out=pW1t[:], in_=pW1t_d[:])
            pW1b = cp.tile([64, 64], BF16, tag="pW1b")
            nc.sync.dma_start(out=pW1b[:], in_=pW1b_d[:])
            b1col = cp.tile([64, 1], F32, tag="b1col")
            nc.sync.dma_start(out=b1col[:], in_=b1col_d[:])
            b2col = cp.tile([64, 1], F32, tag="b2col")
            nc.sync.dma_start(out=b2col[:], in_=b2col_d[:])
            pb1col = cp.tile([64, 1], F32, tag="pb1col")
            nc.sync.dma_start(out=pb1col[:], in_=pb1col_d[:])
            b2pred = cp.tile([128, 1], F32, tag="b2pred")
            nc.sync.dma_start(out=b2pred[:], in_=b2pred_d[:])
            disTu = cp.tile([64, PU], BF16, tag="disTu")
            nc.sync.dma_start(out=disTu[:], in_=disTu_d[:])
            y1ownT = cp.tile([64, PU], F8, tag="y1ownT")
            y2ownT = cp.tile([64, PU], F8, tag="y2ownT")

            def transpose_scatter(srcT, g, dram_out, row0, extra_dep=None):
                """srcT [64, g*128] fp8 -> row-major rows [row0, row0+g*128)."""
                trp = pst.tile([128, 256], F32, tag="trp")
                for q in range(g):
                    # transpose via a regular fp8 matmul: out = srcT_slice^T @ I
                    nc.tensor.matmul(out=trp[:, q * 64:(q + 1) * 64],
                                     lhsT=srcT[:, q * 128:(q + 1) * 128],
                                     rhs=idn8[:64, :64], start=True, stop=True)
                nnm = sb.tile([128, 256], F8, tag="nnm", bufs=3)
                nc.vector.tensor_copy(out=nnm[:, :g * 64], in_=trp[:, :g * 64])
                d = nc.sync.dma_start(
                    out=AP(dram_out[:].tensor, row0 * 64,
                           [[64, 128], [8192, g], [1, 64]]),
                    in_=AP(nnm[:].tensor, nnm[:].offset,
                           [list(nnm[:].ap[0]), [64, g], [1, 64]]),
                )
                return d

            # ================= P1: user projection + y1 table =================
            sc1 = [[], []]  # scatters per half
            for g in range(49):  # user tiles 0..195, groups of 4
                gt = min(4, 49 * 4 - g * 4)
                col0 = g * 4 * 128
                w = 512
                ft = sb.tile([128, 512], BF16, tag="p1_ft", bufs=3)
                nc.sync.dma_start(out=ft[:], in_=featT_d[:, col0:col0 + w])
                et = sb.tile([64, 512], BF16, tag="p1_et", bufs=3)
                nc.sync.dma_start(out=et[:], in_=embT_d[:, col0:col0 + w])
                x0p = ps.tile([64, 512], F32, tag="psA")
                nc.tensor.matmul(out=x0p[:], lhsT=Wuf[:], rhs=ft[:], start=True, stop=True)
                x0s = sb.tile([64, 512], BF16, tag="p1_x0s", bufs=3)
                nc.vector.tensor_add(out=x0s[:], in0=x0p[:], in1=et[:])
                y1p = ps.tile([64, 512], F32, tag="psB")
                nc.tensor.matmul(out=y1p[:], lhsT=W1c[:], rhs=x0s[:], start=True, stop=True)
                nc.vector.tensor_tensor(out=y1ownT[:, col0:col0 + w], in0=y1p[:],
                                        in1=disTu[:, col0:col0 + w], op=ALU.mult)
                d = transpose_scatter(y1ownT[:, col0:col0 + w], 4, ag1_in, col0)
                sc1[0 if g < 25 else 1].append(d)
                # note: group 24 spans tiles 96-99 (both halves); its scatter is
                # rows 12288..12800 -> belongs to half0 rows except 12544+.
                # Keep halves at group granularity: half0 = groups 0..24 covers
                # rows 0..12800 > 12544. So split AG at group boundary instead:
            # AG halves at row granularity matching pi map: half0 rows [0,12544)
            # are fully written once groups 0..24 done (group 24 covers
            # 12288..12800 which includes all of [12288,12544)).

            def allgather(src, r0, r1, dst, o0, scatters):
                cc = nc.gpsimd.collective_compute(
                    "AllGather", ALU.bypass,
                    ins=[src[r0:r1, :]],
                    outs=[dst[o0:o0 + N_CORES * (r1 - r0), :]],
                    replica_groups=[list(range(N_CORES))],
                )
                for s in scatters:
                    add_dep_helper(cc.ins, s.ins, sync=True, reason="AG reads scatters")
                return cc

            cc1a = allgather(ag1_in, 0, P0, y1_t, 0, sc1[0])
            cc1b = allgather(ag1_in, P0, PU, y1_t, UH2, sc1[0] + sc1[1])
            cc1 = [cc1a, cc1b]

            # ================= P1-C: product local chain -> ts table ==========
            sc3c = []
            for g in range(25):  # product tiles, groups of 4 (last has 3)
                gt = min(4, TILES_C - g * 4)
                col0 = PU + g * 4 * 128
                w = gt * 128
                ft = sb.tile([128, 512], BF16, tag="p1c_ft", bufs=3)
                nc.sync.dma_start(out=ft[:, :w], in_=featT_d[:, col0:col0 + w])
                et = sb.tile([64, 512], BF16, tag="p1c_et", bufs=3)
                nc.sync.dma_start(out=et[:, :w], in_=embT_d[:, col0:col0 + w])
                x0p = ps.tile([64, 512], F32, tag="psA")
                nc.tensor.matmul(out=x0p[:, :w], lhsT=Wpf[:], rhs=ft[:, :w], start=True, stop=True)
                x0s = sb.tile([64, 512], BF16, tag="p1c_x0s", bufs=3)
                nc.vector.tensor_add(out=x0s[:, :w], in0=x0p[:, :w], in1=et[:, :w])
                x1p = ps.tile([64, 512], F32, tag="psB")
                nc.tensor.matmul(out=x1p[:, :w], lhsT=W1c[:], rhs=x0s[:, :w], start=True, stop=True)
                x1r = sb.tile([64, 512], BF16, tag="p1c_x1r", bufs=3)
                nc.scalar.activation(out=x1r[:, :w], in_=x1p[:, :w], func=AF.Relu, bias=b1col[:])
                x2p = ps.tile([64, 512], F32, tag="psA")
                nc.tensor.matmul(out=x2p[:, :w], lhsT=W2c[:], rhs=x1r[:, :w], start=True, stop=True)
                x2s = sb.tile([64, 512], BF16, tag="p1c_x2s", bufs=3)
                nc.scalar.activation(out=x2s[:, :w], in_=x2p[:, :w], func=AF.Identity, bias=b2col[:])
                tsp = ps.tile([64, 512], F32, tag="psB")
                nc.tensor.matmul(out=tsp[:, :w], lhsT=pW1b[:], rhs=x2s[:, :w], start=True, stop=True)
                tsc = sb.tile([64, 512], F8, tag="p1c_tsc", bufs=3)
                nc.vector.tensor_copy(out=tsc[:, :w], in_=tsp[:, :w])
                d = transpose_scatter(tsc[:, :w], gt, ag3_in, col0)
                sc3c.append(d)
            cc3c = allgather(ag3_in, PU, SHARD, ts_t, PROD_BASE, sc3c)

            # ================= conv passes =================
            def conv_pass(yt, yprevT, layer, ag_out, cc_dep):
                scatters = [[], []]
                hist = {}
                gidx = 0
                for reg, (tiles, nch, rows_d, cols_d, t0) in enumerate((
                        (TILES_A, nchA, rows4A_d, cols4A_d, 0),
                        (TILES_B, nchB, rows4B_d, cols4B_d, TILES_A))):
                    ng = (tiles + 3) // 4
                    for g in range(ng):
                        gt = min(4, tiles - 4 * g)
                        ncols = gt * nch
                        rw = sb.tile([128, 4 * nchA], I32, tag="cv_rw", bufs=4)
                        drw = nc.sync.dma_start(out=rw[:, :4 * nch],
                                                in_=rows_d[g * 128:(g + 1) * 128, :])
                        cl = sb.tile([128, 4 * nchA], BF16, tag="cv_cl", bufs=4)
                        nc.sync.dma_start(out=cl[:, :4 * nch],
                                          in_=cols_d[g * 128:(g + 1) * 128, :])
                        msg = sb.tile([128, 4 * nchA * 64], F8, tag="cv_msg", bufs=3)
                        gm = nc.gpsimd.indirect_dma_start(
                            out=msg[:, :ncols * 64], out_offset=None,
                            in_=yt[:],
                            in_offset=IndirectOffsetOnAxis(ap=rw[:, :ncols], axis=0),
                        )
                        add_dep_helper(gm.ins, drw.ins, sync=True, reason="gather reads offsets")
                        for _c in cc_dep:
                            add_dep_helper(gm.ins, _c.ins, sync=True, reason="gather after AG")
                        if (gidx - 3) in hist:
                            add_dep_helper(gm.ins, hist[gidx - 3].ins, sync=True,
                                           reason="WAR msg slot reuse")
                        # one-hot tiles: 4 chunks per vector op
                        s4s = []
                        for q in range(0, ncols, 4):
                            nsub = min(4, ncols - q)
                            S4 = sb.tile([128, 512], F8, tag="cv_S4", bufs=20)
                            nc.vector.tensor_tensor(
                                out=_o3(S4[:], nsub),
                                in0=_v3(cl[:, q:q + nsub], nsub, 128),
                                in1=_v3(iota_b[:], nsub, 128, mid_stride=0, inner_stride=1),
                                op=ALU.is_equal,
                            )
                            s4s.append(S4)
                        scp = pssc.tile([64, 512], F32, tag="cv_scp")
                        mm = None
                        for j in range(nch):
                            for t in range(gt):
                                ch = t * nch + j
                                mm = nc.tensor.matmul(
                                    out=scp[:, t * 128:(t + 1) * 128],
                                    lhsT=msg[:, ch * 64:(ch + 1) * 64],
                                    rhs=s4s[ch // 4][:, (ch % 4) * 128:(ch % 4 + 1) * 128],
                                    start=(j == 0), stop=(j == nch - 1),
                                )
                                add_dep_helper(mm.ins, gm.ins, sync=True,
                                               reason="matmul reads gathered msg")
                        hist[gidx] = mm
                        gidx += 1
                        col0 = (t0 + 4 * g) * 128
                        w = gt * 128
                        t1 = sb.tile([64, 512], F32, tag="cv_t1", bufs=3)
                        nc.vector.tensor_add(out=t1[:, :w], in0=scp[:, :w],
                                             in1=yprevT[:, col0:col0 + w])
                        t2 = sb.tile([64, 512], BF16, tag="cv_t2", bufs=3)
                        nc.vector.tensor_tensor(out=t2[:, :w], in0=t1[:, :w],
                                                in1=disTu[:, col0:col0 + w], op=ALU.mult)
                        if layer == 1:
                            x1r = sb.tile([64, 512], BF16, tag="cv_x1r", bufs=3)
                            nc.scalar.activation(out=x1r[:, :w], in_=t2[:, :w],
                                                 func=AF.Relu, bias=b1col[:])
                            y2p = ps.tile([64, 512], F32, tag="psB")
                            nc.tensor.matmul(out=y2p[:, :w], lhsT=W2c[:], rhs=x1r[:, :w],
                                             start=True, stop=True)
                            nc.vector.tensor_tensor(out=y2ownT[:, col0:col0 + w],
                                                    in0=y2p[:, :w],
                                                    in1=disTu[:, col0:col0 + w], op=ALU.mult)
                            d = transpose_scatter(y2ownT[:, col0:col0 + w], gt, ag_out, col0)
                        else:
                            x2s = sb.tile([64, 512], BF16, tag="cv_x2s", bufs=3)
                            nc.scalar.activation(out=x2s[:, :w], in_=t2[:, :w],
                                                 func=AF.Identity, bias=b2col[:])
                            tsp = ps.tile([64, 512], F32, tag="psB")
                            nc.tensor.matmul(out=tsp[:, :w], lhsT=pW1t[:], rhs=x2s[:, :w],
                                             start=True, stop=True)
                            tsu = sb.tile([64, 512], F8, tag="cv_tsu", bufs=3)
                            nc.scalar.activation(out=tsu[:, :w], in_=tsp[:, :w],
                                                 func=AF.Identity, bias=pb1col[:])
                            d = transpose_scatter(tsu[:, :w], gt, ag_out, col0)
                        scatters[reg].append(d)
                return scatters

            cv1 = conv_pass(y1_t, y1ownT, 1, ag2_in, cc1)
            cc2a = allgather(ag2_in, 0, P0, y2_t, 0, cv1[0])
            cc2b = allgather(ag2_in, P0, PU, y2_t, UH2, cv1[0] + cv1[1])
            cv2 = conv_pass(y2_t, y2ownT, 2, ag3_in, [cc2a, cc2b])
            cc3a = allgather(ag3_in, 0, P0, ts_t, 0, cv2[0])
            cc3b = allgather(ag3_in, P0, PU, ts_t, UH2, cv2[0] + cv2[1])
            cc3 = [cc3a, cc3b, cc3c]

            # ================= P7: final pair MLP =================
            offUP_t = cp.tile([128, 2 * NCH], I32, tag="offUP_t")
            doff = nc.sync.dma_start(out=offUP_t[:], in_=offUP_d[:])
            pacc = cp.tile([128, NCH], F32, tag="pacc")
            nneg = 64 - npos
            p7h = {}
            nblk = (NCH // 8 + 3) // 4  # 31 blocks of up to 4 gchunks
            for b in range(nblk):
                g0 = b * 4
                gn = min(4, NCH // 8 - g0)
                ncols16 = gn * 16
                tUP = sb.tile([128, 4096], F8, tag="p7_tUP", bufs=2)
                gt_ = nc.gpsimd.indirect_dma_start(
                    out=tUP[:, :ncols16 * 64], out_offset=None, in_=ts_t[:],
                    in_offset=IndirectOffsetOnAxis(
                        ap=offUP_t[:, 16 * g0:16 * g0 + ncols16], axis=0),
                )
                add_dep_helper(gt_.ins, doff.ins, sync=True, reason="gather reads offsets")
                for _c in cc3:
                    add_dep_helper(gt_.ins, _c.ins, sync=True, reason="gather after AG3")
                if (b - 2) in p7h:
                    add_dep_helper(gt_.ins, p7h[b - 2].ins, sync=True,
                                   reason="WAR tUP slot reuse")
                h8 = sb.tile([128, 2048], BF16, tag="p7_h8", bufs=2)
                a8 = nc.vector.tensor_tensor(
                    out=AP(h8[:].tensor, h8[:].offset,
                           [list(h8[:].ap[0]), [512, gn], [1, 512]]),
                    in0=AP(tUP[:].tensor, tUP[:].offset,
                           [list(tUP[:].ap[0]), [1024, gn], [1, 512]]),
                    in1=AP(tUP[:].tensor, tUP[:].offset + 512,
                           [list(tUP[:].ap[0]), [1024, gn], [1, 512]]),
                    op=ALU.add,
                )
                add_dep_helper(a8.ins, gt_.ins, sync=True, reason="reads tUP")
                p7h[b] = a8
                hr = sb.tile([128, 2048], BF16, tag="p7_hr", bufs=2)
                nc.scalar.activation(out=hr[:, :gn * 512], in_=h8[:, :gn * 512], func=AF.Relu)
                redp = sb.tile([128, 32], F32, tag="p7_redp", bufs=3)
                redn = sb.tile([128, 32], F32, tag="p7_redn", bufs=3)
                ncr = gn * 8
                if npos > 0:
                    nc.vector.tensor_reduce(
                        out=redp[:, :ncr],
                        in_=AP(hr[:].tensor, hr[:].offset,
                               [list(hr[:].ap[0]), [64, ncr], [1, npos]]),
                        axis=mybir.AxisListType.X, op=ALU.add,
                    )
                else:
                    nc.gpsimd.memset(redp[:, :ncr], 0.0)
                if nneg > 0:
                    nc.vector.tensor_reduce(
                        out=redn[:, :ncr],
                        in_=AP(hr[:].tensor, hr[:].offset + npos,
                               [list(hr[:].ap[0]), [64, ncr], [1, nneg]]),
                        axis=mybir.AxisListType.X, op=ALU.add,
                    )
                else:
                    nc.gpsimd.memset(redn[:, :ncr], 0.0)
                df = sb.tile([128, 32], F32, tag="p7_df", bufs=3)
                nc.vector.tensor_tensor(out=df[:, :ncr], in0=redp[:, :ncr],
                                        in1=redn[:, :ncr], op=ALU.subtract)
                sg = sb.tile([128, 32], F32, tag="p7_sg", bufs=3)
                nc.scalar.activation(out=sg[:, :ncr], in_=df[:, :ncr],
                                     func=AF.Sigmoid, bias=b2pred[:], scale=float(inv_g))
                nc.vector.tensor_scalar_mul(out=pacc[:, g0 * 8:g0 * 8 + ncr],
                                            in0=sg[:, :ncr], scalar1=5.0)
            nc.sync.dma_start(out=preds_d[:], in_=pacc[:])

    _split_sync_waits(nc)
    return nc


# --------------------------------------------------------------------------
# runner
# --------------------------------------------------------------------------
def _run(inputs, trace=False):
    per_core, shared, nchA, nchB, npos, inv_g = _prepare(inputs)
    nc = build_program(nchA, nchB, npos, inv_g)
    in_maps = []
    for c in range(N_CORES):
        m = dict(shared)
        m.update(per_core[c])
        in_maps.append({k: np.ascontiguousarray(v) for k, v in m.items()})
    res = run_bass_kernel_spmd(nc, in_maps, core_ids=list(range(N_CORES)), trace=trace)
    out = np.zeros(NE, np.float32)
    el = np.arange(EPT)
    for c in range(N_CORES):
        pc = res.results[c]["preds"]
        out[c * EPT + el] = pc[el % 128, el // 128]
    return out, res.exec_time_ns


def kernel(**inputs):
    out, _ = _run(inputs, trace=False)
    return out


# revision 7
# speedup vs baseline: 1.2972x; 1.2972x over previous
"""Trainium2 Bass kernel for the bipartite GNN recommender (8 NeuronCores).

Redesigned layout (v2):
- Node j -> core j%8. Per-core user rows l=j//8 in [0,25088), products
  l=25088+(p//8) in [25088,37760). Graph edges only touch nodes <200000
  (the reference never offsets prod_idx), so products are self-loop-only
  and their whole chain (proj->conv1->conv2->ts) is computed locally in
  phase P1 with zero collective traffic.
- Conv tables are block-laid per (half, core): user table row for node j:
  l<12544 -> c*12544+l, else 100352+c*12544+(l-12544). One AllGather per
  half, triggered as soon as that half's tiles are produced (overlaps
  compute). Products in ts table at 200704+c*12672+(p//8).
- Scatter segment-sum via transposed one-hot matmuls: stationary = the
  64-col message tile (half the LDWEIGHTS cost), stream = the one-hot,
  accumulate [64,128] per tile in one [64,512] PSUM bank per group.
- Self-term from a persistent SBUF copy of the core's own table slice
  (no indirect gather), dis applied via a persistent [64,25088] bf16
  broadcast table, biases via per-partition activation bias columns.
- Final pair-MLP: |W2| folded into the ts tables (pos/neg dim split on
  host), so per edge: gather t,s -> add -> relu -> two strided reduces
  -> subtract -> sigmoid(scale)*5.
"""
import ml_dtypes
import numpy as np

from concourse import bass, mybir, tile
from concourse.bass import AP, IndirectOffsetOnAxis
from concourse.bass_utils import run_bass_kernel_spmd
from concourse.masks import make_identity
from concourse.tile import add_dep_helper

F32 = mybir.dt.float32
BF16 = mybir.dt.bfloat16
F8 = mybir.dt.float8e4
I32 = mybir.dt.int32

AF = mybir.ActivationFunctionType
ALU = mybir.AluOpType

N_CORES = 8
NU, NP, NE = 200000, 100000, 1000000
SHARD = 37760
PU = 25088            # user rows per core
PC = 12672            # product rows per core
P0 = 12544            # rows per user half per core
UH2 = 8 * P0          # 100352
PROD_BASE = 2 * UH2   # 200704
TAB = PROD_BASE + 8 * PC  # 302080
TILES_A, TILES_B, TILES_C = 98, 98, 99
EPT = NE // N_CORES
NCH = 984             # pred output cols (125000 edges -> 977, pad to mult of 8)


# --------------------------------------------------------------------------
# legalization: this walrus build allows at most 1 sync wait per instruction
# --------------------------------------------------------------------------
def _split_sync_waits(nc, max_waits=1):
    import bass_rust
    for bb in nc.main_func.blocks:
        out = []
        for inst in bb.instructions:
            si = inst.sync_info
            if si is not None and si.on_wait is not None and len(si.on_wait) > max_waits:
                waits = list(si.on_wait)
                keep, extra = waits[-max_waits:], waits[:-max_waits]
                while extra:
                    chunk, extra = extra[:max_waits], extra[max_waits:]
                    nop = bass_rust.InstNoOp(name=f"I-{nc.next_id()}", ins=[], outs=[])
                    nop.engine = inst.engine
                    nop.bass_nofuse = True
                    nop.sync_info = mybir.SyncInfo(on_wait=chunk, on_update=[])
                    nc.register_instruction(nop, overwrite=True)
                    out.append(nop)
                si.on_wait = keep
            out.append(inst)
        del bb.instructions[:]
        for i in out:
            bb.add_instruction(i)


# --------------------------------------------------------------------------
# host-side sharding / layout prep
# --------------------------------------------------------------------------
def _pi_user(j):
    j = np.asarray(j, np.int64)
    c, l = j % 8, j // 8
    return np.where(l < P0, c * P0 + l, UH2 + c * P0 + (l - P0)).astype(np.int32)


def _pi_prod(p):
    p = np.asarray(p, np.int64)
    return (PROD_BASE + (p % 8) * PC + p // 8).astype(np.int32)


def _tile_blocks(l_sorted, vals_rows, n_tiles, K, t0):
    """Edges sorted by local dest row -> per-tile padded (rows, cols)."""
    t = (l_sorted >> 7) - t0
    start = np.searchsorted(t, np.arange(n_tiles))
    pos = np.arange(len(t)) - start[t]
    assert len(t) == 0 or pos.max() < K, "tile overflow"
    rows = np.zeros((n_tiles, 128, K // 128), np.int32)
    cols = np.full((n_tiles, 128, K // 128), -1.0, np.float32)
    rows[t, pos % 128, pos // 128] = vals_rows
    cols[t, pos % 128, pos // 128] = (l_sorted & 127).astype(np.float32)
    return rows, cols


def _groupify(arr3, nch):
    """[tiles, 128, nch] -> [ngroups*128, 4*nch] (4 tiles side by side)."""
    tiles = arr3.shape[0]
    ng = (tiles + 3) // 4
    out = np.zeros((ng * 128, 4 * nch), arr3.dtype)
    if arr3.dtype != np.int32:
        out[:] = -1.0
    for g in range(ng):
        for t in range(min(4, tiles - 4 * g)):
            out[g * 128:(g + 1) * 128, t * nch:(t + 1) * nch] = arr3[4 * g + t]
    return out


def _prepare(inputs):
    ei = np.asarray(inputs["edge_index"])
    u_idx = ei[0].astype(np.int64)
    p_idx = ei[1].astype(np.int64)

    # directed messages: src -> dst; both endpoints are node ids < 200000
    src = np.concatenate([u_idx, p_idx])
    dst = np.concatenate([p_idx, u_idx])
    core = (dst % 8).astype(np.int64)
    l = (dst // 8).astype(np.int64)
    src_pi = _pi_user(src)

    order = np.argsort(core * (1 << 32) + l, kind="stable")
    core_s, l_s, srcpi_s = core[order], l[order], src_pi[order]
    core_starts = np.searchsorted(core_s, np.arange(N_CORES + 1))

    cnt = np.bincount(core_s * 256 + (l_s >> 7), minlength=N_CORES * 256).reshape(N_CORES, 256)
    K_A = max(128, int(np.ceil(cnt[:, :TILES_A].max() / 128)) * 128)
    K_B = max(128, int(np.ceil(cnt[:, TILES_A:TILES_A + TILES_B].max() / 128)) * 128)
    nchA, nchB = K_A // 128, K_B // 128

    fw = np.asarray(inputs["user_features"], np.float32)
    pw = np.asarray(inputs["product_features"], np.float32)
    ue = np.asarray(inputs["user_emb"], np.float32)
    pe = np.asarray(inputs["product_emb"], np.float32)
    b_uf = np.asarray(inputs["b_uf"], np.float32)
    b_pf = np.asarray(inputs["b_pf"], np.float32)

    pi_u = _pi_user(u_idx)
    pi_p = _pi_prod(p_idx)

    # pred-MLP folding: permute hidden dims so W2>=0 dims come first,
    # scale W1 columns (and pb1) by |W2|*G, recover with sigmoid scale 1/G.
    W1 = np.asarray(inputs["pred_W1"], np.float32)     # [128, 64]
    w2 = np.asarray(inputs["pred_W2"], np.float32).reshape(64)
    pb1 = np.asarray(inputs["pred_b1"], np.float32)
    perm = np.argsort(w2 < 0, kind="stable")           # positives first
    npos = int((w2 >= 0).sum())
    aw = np.abs(w2[perm])
    amax = max(aw.max(), 1e-30)
    G = 1.0 / amax
    colscale = aw * G                                  # in (0, 1]
    W1s = W1[:, perm] * colscale[None, :]
    pb1s = pb1[perm] * colscale
    inv_g = float(amax)                                # sigmoid scale

    per_core = []
    for c in range(N_CORES):
        s0, s1 = core_starts[c], core_starts[c + 1]
        lc, sc = l_s[s0:s1], srcpi_s[s0:s1]
        mA = lc < TILES_A * 128
        rowsA, colsA = _tile_blocks(lc[mA], sc[mA], TILES_A, K_A, 0)
        rowsB, colsB = _tile_blocks(lc[~mA], sc[~mA], TILES_B, K_B, TILES_A)
        rows4A = _groupify(rowsA, nchA)
        cols4A = _groupify(colsA, nchA)
        rows4B = _groupify(rowsB, nchB)
        cols4B = _groupify(colsB, nchB)

        featT = np.zeros((128, SHARD), np.float32)
        embT = np.zeros((64, SHARD), np.float32)
        featT[:, :25000] = fw[c::8].T
        featT[:, 25088:37588] = pw[c::8].T
        embT[:, :25000] = ue[c::8].T + b_uf[:, None]
        embT[:, 25088:37588] = pe[c::8].T + b_pf[:, None]
        embT[:, 25000:25088] = b_uf[:, None]
        embT[:, 37588:] = b_pf[:, None]

        deg = np.bincount(lc, minlength=PU).astype(np.float32)
        dis = 1.0 / np.sqrt(deg + 1.0)
        disTu = np.tile(dis[None, :], (64, 1)).astype(ml_dtypes.bfloat16)

        e0 = c * EPT
        offU = np.zeros((128, NCH), np.int32)
        offP = np.zeros((128, NCH), np.int32)
        el = np.arange(EPT)
        offU[el % 128, el // 128] = pi_u[e0:e0 + EPT]
        offP[el % 128, el // 128] = pi_p[e0:e0 + EPT]
        offUP = np.zeros((128, 2 * NCH), np.int32)
        for g in range(NCH // 8):
            offUP[:, 16 * g:16 * g + 8] = offU[:, 8 * g:8 * g + 8]
            offUP[:, 16 * g + 8:16 * g + 16] = offP[:, 8 * g:8 * g + 8]

        per_core.append(dict(
            featT=featT.astype(ml_dtypes.bfloat16), embT=embT.astype(ml_dtypes.bfloat16),
            disTu=disTu, rows4A=rows4A, cols4A=cols4A, rows4B=rows4B, cols4B=cols4B,
            offUP=offUP,
        ))

    shared = dict(
        Wuf=np.asarray(inputs["W_uf"], np.float32).astype(ml_dtypes.bfloat16),
        Wpf=np.asarray(inputs["W_pf"], np.float32).astype(ml_dtypes.bfloat16),
        W1c=np.asarray(inputs["conv1_W"], np.float32).astype(ml_dtypes.bfloat16),
        W2c=np.asarray(inputs["conv2_W"], np.float32).astype(ml_dtypes.bfloat16),
        pW1t=np.ascontiguousarray(W1s[:64]).astype(ml_dtypes.bfloat16),
        pW1b=np.ascontiguousarray(W1s[64:]).astype(ml_dtypes.bfloat16),
        b1col=np.asarray(inputs["conv1_b"], np.float32).reshape(64, 1),
        b2col=np.asarray(inputs["conv2_b"], np.float32).reshape(64, 1),
        pb1col=pb1s.reshape(64, 1).astype(np.float32),
        b2pred=np.full((128, 1), float(np.asarray(inputs["pred_b2"]).reshape(())), np.float32),
    )
    return per_core, shared, nchA, nchB, npos, inv_g


# --------------------------------------------------------------------------
# numpy simulator of the device program (for host-side validation only)
# --------------------------------------------------------------------------
def _simulate(inputs):
    f8 = lambda x: np.asarray(x, np.float32).astype(ml_dtypes.float8_e4m3).astype(np.float32)
    bf = lambda x: np.asarray(x, np.float32).astype(ml_dtypes.bfloat16).astype(np.float32)
    per_core, shared, nchA, nchB, npos, inv_g = _prepare(inputs)
    Wuf, Wpf = bf(shared["Wuf"]), bf(shared["Wpf"])
    W1c, W2c = bf(shared["W1c"]), bf(shared["W2c"])
    pW1t, pW1b = bf(shared["pW1t"]), bf(shared["pW1b"])
    b1, b2 = shared["b1col"][:, 0], shared["b2col"][:, 0]
    pb1 = shared["pb1col"][:, 0]

    y1_t = np.zeros((PROD_BASE, 64), np.float32)
    ts_t = np.zeros((TAB, 64), np.float32)
    y1ownT, disT, featsT, embsT = [], [], [], []
    for c in range(N_CORES):
        pc = per_core[c]
        ft, et = bf(pc["featT"]), bf(pc["embT"])
        dis = bf(pc["disTu"])[0]  # [PU]
        x0 = bf(ft.T @ Wuf + et.T)              # [SHARD, 64] (user cols valid)
        y1 = f8((x0[:PU] @ W1c) * dis[:, None])
        y1ownT.append(y1)
        disT.append(dis)
        featsT.append(ft)
        embsT.append(et)
        for h, base in ((0, 0), (1, UH2)):
            y1_t[base + c * P0: base + (c + 1) * P0] = y1[h * P0:(h + 1) * P0]
        # region C local chain
        x0c = bf(ft[:, PU:].T @ Wpf + et[:, PU:].T)
        x1c = np.maximum(bf(x0c @ W1c) + b1, 0.0)
        x2c = bf(bf(x1c) @ W2c) + b2
        tsc = f8(bf(x2c) @ pW1b)
        ts_t[PROD_BASE + c * PC: PROD_BASE + (c + 1) * PC] = tsc

    def conv(y_t, layer):
        y2_t = np.zeros((PROD_BASE, 64), np.float32)
        outs = []
        for c in range(N_CORES):
            pc = per_core[c]
            dis = disT[c]
            aggT = np.zeros((64, PU), np.float32)
            for reg, (tiles, nch, r4, c4, t0) in enumerate((
                    (TILES_A, nchA, pc["rows4A"], pc["cols4A"], 0),
                    (TILES_B, nchB, pc["rows4B"], pc["cols4B"], TILES_A))):
                ng = (tiles + 3) // 4
                for g in range(ng):
                    gt = min(4, tiles - 4 * g)
                    rw = r4[g * 128:(g + 1) * 128]
                    cl = np.asarray(c4[g * 128:(g + 1) * 128], np.float32)
                    msg = f8(y_t[rw])           # [128, 4nch, 64]
                    for t in range(gt):
                        acc = np.zeros((64, 128), np.float32)
                        for j in range(nch):
                            ch = t * nch + j
                            S = (cl[:, ch:ch + 1] == np.arange(128)[None, :]).astype(np.float32)
                            acc += msg[:, ch].T @ S
                        tile_i = t0 + 4 * g + t
                        aggT[:, tile_i * 128:(tile_i + 1) * 128] = acc
            own = y1ownT[c] if layer == 1 else yown2[c]
            agg = aggT.T + own
            x = bf(agg * dis[:, None])
            if layer == 1:
                x1 = bf(np.maximum(x + b1, 0.0))
                y2 = f8(bf(x1 @ W2c) * dis[:, None])
                outs.append(y2)
                for h, base in ((0, 0), (1, UH2)):
                    y2_t[base + c * P0: base + (c + 1) * P0] = y2[h * P0:(h + 1) * P0]
            else:
                x2 = bf(x + b2)
                ts = f8(bf(x2 @ pW1t) + pb1)
                outs.append(ts)
                for h, base in ((0, 0), (1, UH2)):
                    ts_t[base + c * P0: base + (c + 1) * P0] = ts[h * P0:(h + 1) * P0]
        return y2_t, outs

    yown2 = None
    y2_t, yown2 = conv(y1_t, 1)
    _, _ = conv(y2_t, 2)

    # P7
    out = np.zeros(NE, np.float32)
    ei = np.asarray(inputs["edge_index"])
    pi_u = _pi_user(ei[0].astype(np.int64))
    pi_p = _pi_prod(ei[1].astype(np.int64))
    t = ts_t[pi_u]
    s = ts_t[pi_p]
    h = bf(np.maximum(bf(t + s), 0.0))
    logit = h[:, :npos].sum(1) - h[:, npos:].sum(1)
    z = logit * inv_g + float(np.asarray(inputs["pred_b2"]).reshape(()))
    out[:] = 5.0 / (1.0 + np.exp(-z))
    return out


# --------------------------------------------------------------------------
# device program
# --------------------------------------------------------------------------
def _v3(ap, mid, inner, mid_stride=None, inner_stride=0):
    a = ap.ap
    ms = a[1][0] if mid_stride is None else mid_stride
    return AP(ap.tensor, ap.offset, [list(a[0]), [ms, mid], [inner_stride, inner]])


def _o3(ap, nsub):
    return AP(ap.tensor, ap.offset, [list(ap.ap[0]), [128, nsub], [1, 128]])


def build_program(nchA, nchB, npos, inv_g):
    nc = bass.Bass("TRN2", target_bir_lowering=False, debug=False, num_devices=N_CORES)

    dp = nc.declare_dram_parameter
    featT_d = dp("featT", [128, SHARD], BF16, isOutput=False)
    embT_d = dp("embT", [64, SHARD], BF16, isOutput=False)
    disTu_d = dp("disTu", [64, PU], BF16, isOutput=False)
    rows4A_d = dp("rows4A", [25 * 128, 4 * nchA], I32, isOutput=False)
    cols4A_d = dp("cols4A", [25 * 128, 4 * nchA], F32, isOutput=False)
    rows4B_d = dp("rows4B", [25 * 128, 4 * nchB], I32, isOutput=False)
    cols4B_d = dp("cols4B", [25 * 128, 4 * nchB], F32, isOutput=False)
    offUP_d = dp("offUP", [128, 2 * NCH], I32, isOutput=False)
    Wuf_d = dp("Wuf", [128, 64], BF16, isOutput=False)
    Wpf_d = dp("Wpf", [128, 64], BF16, isOutput=False)
    W1c_d = dp("W1c", [64, 64], BF16, isOutput=False)
    W2c_d = dp("W2c", [64, 64], BF16, isOutput=False)
    pW1t_d = dp("pW1t", [64, 64], BF16, isOutput=False)
    pW1b_d = dp("pW1b", [64, 64], BF16, isOutput=False)
    b1col_d = dp("b1col", [64, 1], F32, isOutput=False)
    b2col_d = dp("b2col", [64, 1], F32, isOutput=False)
    pb1col_d = dp("pb1col", [64, 1], F32, isOutput=False)
    b2pred_d = dp("b2pred", [128, 1], F32, isOutput=False)
    preds_d = dp("preds", [128, NCH], F32, isOutput=True)

    with tile.TileContext(nc) as tc:
        with tc.tile_pool(name="const", bufs=1) as cp, \
             tc.tile_pool(name="sb", bufs=3) as sb, \
             tc.tile_pool(name="ps", bufs=2, space="PSUM") as ps, \
             tc.tile_pool(name="pssc", bufs=2, space="PSUM") as pssc, \
             tc.tile_pool(name="pst", bufs=2, space="PSUM") as pst:

            def reg_dge(h):
                mloc = nc.lookup_mloc(h)
                if mloc.table_entry_id is None:
                    mloc.table_entry_id = len(nc.dge_table) + 1
                    nc.dge_table.append(mloc.name)
                return h

            ag1_in = reg_dge(nc.dram_tensor("ag1_in", [PU, 64], F8))
            ag2_in = reg_dge(nc.dram_tensor("ag2_in", [PU, 64], F8))
            ag3_in = reg_dge(nc.dram_tensor("ag3_in", [SHARD, 64], F8))
            y1_t = reg_dge(nc.dram_tensor("y1_t", [PROD_BASE, 64], F8, addr_space="Shared"))
            y2_t = reg_dge(nc.dram_tensor("y2_t", [PROD_BASE, 64], F8, addr_space="Shared"))
            ts_t = reg_dge(nc.dram_tensor("ts_t", [TAB, 64], F8, addr_space="Shared"))

            # ---- constants ----
            iota_i = cp.tile([128, 128], I32, tag="iota_i")
            nc.gpsimd.iota(iota_i[:], [[1, 128]], channel_multiplier=0)
            iota_b = cp.tile([128, 128], BF16, tag="iota_b")
            nc.vector.tensor_copy(out=iota_b[:], in_=iota_i[:])
            idn = cp.tile([128, 128], F32, tag="idn")
            make_identity(nc, idn[:])
            idn8 = cp.tile([128, 128], F8, tag="idn8")
            nc.vector.tensor_copy(out=idn8[:], in_=idn[:])

            Wuf = cp.tile([128, 64], BF16, tag="Wuf")
            nc.sync.dma_start(out=Wuf[:], in_=Wuf_d[:])
            Wpf = cp.tile([128, 64], BF16, tag="Wpf")
            nc.sync.dma_start(out=Wpf[:], in_=Wpf_d[:])
            W1c = cp.tile([64, 64], BF16, tag="W1c")
            nc.sync.dma_start(out=W1c[:], in_=W1c_d[:])
            W2c = cp.tile([64, 64], BF16, tag="W2c")
            nc.sync.dma_start(out=W2c[:], in_=W2c_d[:])
            pW1t = cp.tile([64, 64], BF16, tag="pW1t")
            nc.sync.dma_start(out=pW1t[:], in_=pW1t_d[:])
            pW1b = cp.tile([64, 64], BF16, tag="pW1b")
            nc.sync.dma_start(out=pW1b[:], in_=pW1b_d[:])
            b1col = cp.tile([64, 1], F32, tag="b1col")
            nc.sync.dma_start(out=b1col[:], in_=b1col_d[:])
            b2col = cp.tile([64, 1], F32, tag="b2col")
            nc.sync.dma_start(out=b2col[:], in_=b2col_d[:])
            pb1col = cp.tile([64, 1], F32, tag="pb1col")
            nc.sync.dma_start(out=pb1col[:], in_=pb1col_d[:])
            b2pred = cp.tile([128, 1], F32, tag="b2pred")
            nc.sync.dma_start(out=b2pred[:], in_=b2pred_d[:])
            disTu = cp.tile([64, PU], BF16, tag="disTu")
            nc.sync.dma_start(out=disTu[:], in_=disTu_d[:])
            y1ownT = cp.tile([64, PU], F8, tag="y1ownT")
            y2ownT = cp.tile([64, PU], F8, tag="y2ownT")

            def transpose_scatter(srcT, g, dram_out, row0, extra_dep=None):
                """srcT [64, g*128] fp8 -> row-major rows [row0, row0+g*128)."""
                trp = pst.tile([128, 256], F32, tag="trp")
                for q in range(g):
                    # transpose via a regular fp8 matmul: out = srcT_slice^T @ I
                    nc.tensor.matmul(out=trp[:, q * 64:(q + 1) * 64],
                                     lhsT=srcT[:, q * 128:(q + 1) * 128],
                                     rhs=idn8[:64, :64], start=True, stop=True)
                nnm = sb.tile([128, 256], F8, tag="nnm", bufs=3)
                nc.scalar.activation(out=nnm[:, :g * 64], in_=trp[:, :g * 64], func=AF.Copy)
                d = nc.sync.dma_start(
                    out=AP(dram_out[:].tensor, row0 * 64,
                           [[64, 128], [8192, g], [1, 64]]),
                    in_=AP(nnm[:].tensor, nnm[:].offset,
                           [list(nnm[:].ap[0]), [64, g], [1, 64]]),
                )
                return d

            # ================= P1: user projection + y1 table =================
            sc1 = [[], []]  # scatters per half
            for g in range(49):  # user tiles 0..195, groups of 4
                gt = min(4, 49 * 4 - g * 4)
                col0 = g * 4 * 128
                w = 512
                ft = sb.tile([128, 512], BF16, tag="p1_ft", bufs=3)
                nc.sync.dma_start(out=ft[:], in_=featT_d[:, col0:col0 + w])
                et = sb.tile([64, 512], BF16, tag="p1_et", bufs=3)
                nc.sync.dma_start(out=et[:], in_=embT_d[:, col0:col0 + w])
                x0p = ps.tile([64, 512], F32, tag="psA")
                nc.tensor.matmul(out=x0p[:], lhsT=Wuf[:], rhs=ft[:], start=True, stop=True)
                x0s = sb.tile([64, 512], BF16, tag="p1_x0s", bufs=3)
                nc.vector.tensor_add(out=x0s[:], in0=x0p[:], in1=et[:])
                y1p = ps.tile([64, 512], F32, tag="psB")
                nc.tensor.matmul(out=y1p[:], lhsT=W1c[:], rhs=x0s[:], start=True, stop=True)
                nc.vector.tensor_tensor(out=y1ownT[:, col0:col0 + w], in0=y1p[:],
                                        in1=disTu[:, col0:col0 + w], op=ALU.mult)
                d = transpose_scatter(y1ownT[:, col0:col0 + w], 4, ag1_in, col0)
                sc1[0 if g < 25 else 1].append(d)
                # note: group 24 spans tiles 96-99 (both halves); its scatter is
                # rows 12288..12800 -> belongs to half0 rows except 12544+.
                # Keep halves at group granularity: half0 = groups 0..24 covers
                # rows 0..12800 > 12544. So split AG at group boundary instead:
            # AG halves at row granularity matching pi map: half0 rows [0,12544)
            # are fully written once groups 0..24 done (group 24 covers
            # 12288..12800 which includes all of [12288,12544)).

            def allgather(src, r0, r1, dst, o0, scatters):
                cc = nc.gpsimd.collective_compute(
                    "AllGather", ALU.bypass,
                    ins=[src[r0:r1, :]],
                    outs=[dst[o0:o0 + N_CORES * (r1 - r0), :]],
                    replica_groups=[list(range(N_CORES))],
                )
                for s in scatters:
                    add_dep_helper(cc.ins, s.ins, sync=True, reason="AG reads scatters")
                return cc

            cc1a = allgather(ag1_in, 0, P0, y1_t, 0, sc1[0])
            cc1b = allgather(ag1_in, P0, PU, y1_t, UH2, sc1[0] + sc1[1])
            cc1 = [cc1a, cc1b]

            # ================= P1-C: product local chain -> ts table ==========
            sc3c = []
            for g in range(25):  # product tiles, groups of 4 (last has 3)
                gt = min(4, TILES_C - g * 4)
                col0 = PU + g * 4 * 128
                w = gt * 128
                ft = sb.tile([128, 512], BF16, tag="p1c_ft", bufs=3)
                nc.sync.dma_start(out=ft[:, :w], in_=featT_d[:, col0:col0 + w])
                et = sb.tile([64, 512], BF16, tag="p1c_et", bufs=3)
                nc.sync.dma_start(out=et[:, :w], in_=embT_d[:, col0:col0 + w])
                x0p = ps.tile([64, 512], F32, tag="psA")
                nc.tensor.matmul(out=x0p[:, :w], lhsT=Wpf[:], rhs=ft[:, :w], start=True, stop=True)
                x0s = sb.tile([64, 512], BF16, tag="p1c_x0s", bufs=3)
                nc.vector.tensor_add(out=x0s[:, :w], in0=x0p[:, :w], in1=et[:, :w])
                x1p = ps.tile([64, 512], F32, tag="psB")
                nc.tensor.matmul(out=x1p[:, :w], lhsT=W1c[:], rhs=x0s[:, :w], start=True, stop=True)
                x1r = sb.tile([64, 512], BF16, tag="p1c_x1r", bufs=3)
                nc.scalar.activation(out=x1r[:, :w], in_=x1p[:, :w], func=AF.Relu, bias=b1col[:])
                x2p = ps.tile([64, 512], F32, tag="psA")
                nc.tensor.matmul(out=x2p[:, :w], lhsT=W2c[:], rhs=x1r[:, :w], start=True, stop=True)
                x2s = sb.tile([64, 512], BF16, tag="p1c_x2s", bufs=3)
                nc.scalar.activation(out=x2s[:, :w], in_=x2p[:, :w], func=AF.Identity, bias=b2col[:])
                tsp = ps.tile([64, 512], F32, tag="psB")
                nc.tensor.matmul(out=tsp[:, :w], lhsT=pW1b[:], rhs=x2s[:, :w], start=True, stop=True)
                tsc = sb.tile([64, 512], F8, tag="p1c_tsc", bufs=3)
                nc.vector.tensor_copy(out=tsc[:, :w], in_=tsp[:, :w])
                d = transpose_scatter(tsc[:, :w], gt, ag3_in, col0)
                sc3c.append(d)

            # ================= conv passes =================
            def conv_pass(yt, yprevT, layer, ag_out, cc_dep, hooks=None):
                scatters = [[], []]
                hist = {}
                gidx = 0
                for reg, (tiles, nch, rows_d, cols_d, t0) in enumerate((
                        (TILES_A, nchA, rows4A_d, cols4A_d, 0),
                        (TILES_B, nchB, rows4B_d, cols4B_d, TILES_A))):
                    ng = (tiles + 3) // 4
                    for g in range(ng):
                        gt = min(4, tiles - 4 * g)
                        ncols = gt * nch
                        rw = sb.tile([128, 4 * nchA], I32, tag="cv_rw", bufs=4)
                        drw = nc.sync.dma_start(out=rw[:, :4 * nch],
                                                in_=rows_d[g * 128:(g + 1) * 128, :])
                        cl = sb.tile([128, 4 * nchA], F32, tag="cv_cl", bufs=4)
                        nc.sync.dma_start(out=cl[:, :4 * nch],
                                          in_=cols_d[g * 128:(g + 1) * 128, :])
                        msg = sb.tile([128, 4 * nchA * 64], F8, tag="cv_msg", bufs=3)
                        gm = nc.gpsimd.indirect_dma_start(
                            out=msg[:, :ncols * 64], out_offset=None,
                            in_=yt[:],
                            in_offset=IndirectOffsetOnAxis(ap=rw[:, :ncols], axis=0),
                        )
                        add_dep_helper(gm.ins, drw.ins, sync=True, reason="gather reads offsets")
                        for _c in cc_dep:
                            add_dep_helper(gm.ins, _c.ins, sync=True, reason="gather after AG")
                        if (gidx - 3) in hist:
                            add_dep_helper(gm.ins, hist[gidx - 3].ins, sync=True,
                                           reason="WAR msg slot reuse")
                        # one-hot tiles: 8 chunks per vector op
                        s4s = []
                        for q in range(0, ncols, 8):
                            nsub = min(8, ncols - q)
                            S4 = sb.tile([128, 1024], F8, tag="cv_S4", bufs=10)
                            nc.vector.tensor_tensor(
                                out=_o3(S4[:], nsub),
                                in0=_v3(cl[:, q:q + nsub], nsub, 128),
                                in1=_v3(iota_b[:], nsub, 128, mid_stride=0, inner_stride=1),
                                op=ALU.is_equal,
                            )
                            s4s.append(S4)
                        scp = pssc.tile([64, 512], F32, tag="cv_scp")
                        mm = None
                        for j in range(nch):
                            for t in range(gt):
                                ch = t * nch + j
                                mm = nc.tensor.matmul(
                                    out=scp[:, t * 128:(t + 1) * 128],
                                    lhsT=msg[:, ch * 64:(ch + 1) * 64],
                                    rhs=s4s[ch // 8][:, (ch % 8) * 128:(ch % 8 + 1) * 128],
                                    start=(j == 0), stop=(j == nch - 1),
                                )
                                add_dep_helper(mm.ins, gm.ins, sync=True,
                                               reason="matmul reads gathered msg")
                        hist[gidx] = mm
                        gidx += 1
                        col0 = (t0 + 4 * g) * 128
                        w = gt * 128
                        t1 = sb.tile([64, 512], F32, tag="cv_t1", bufs=3)
                        nc.vector.tensor_add(out=t1[:, :w], in0=scp[:, :w],
                                             in1=yprevT[:, col0:col0 + w])
                        t2 = sb.tile([64, 512], BF16, tag="cv_t2", bufs=3)
                        nc.vector.tensor_tensor(out=t2[:, :w], in0=t1[:, :w],
                                                in1=disTu[:, col0:col0 + w], op=ALU.mult)
                        if layer == 1:
                            x1r = sb.tile([64, 512], BF16, tag="cv_x1r", bufs=3)
                            nc.scalar.activation(out=x1r[:, :w], in_=t2[:, :w],
                                                 func=AF.Relu, bias=b1col[:])
                            y2p = ps.tile([64, 512], F32, tag="psB")
                            nc.tensor.matmul(out=y2p[:, :w], lhsT=W2c[:], rhs=x1r[:, :w],
                                             start=True, stop=True)
                            nc.vector.tensor_tensor(out=y2ownT[:, col0:col0 + w],
                                                    in0=y2p[:, :w],
                                                    in1=disTu[:, col0:col0 + w], op=ALU.mult)
                            d = transpose_scatter(y2ownT[:, col0:col0 + w], gt, ag_out, col0)
                        else:
                            x2s = sb.tile([64, 512], BF16, tag="cv_x2s", bufs=3)
                            nc.scalar.activation(out=x2s[:, :w], in_=t2[:, :w],
                                                 func=AF.Identity, bias=b2col[:])
                            tsp = ps.tile([64, 512], F32, tag="psB")
                            nc.tensor.matmul(out=tsp[:, :w], lhsT=pW1t[:], rhs=x2s[:, :w],
                                             start=True, stop=True)
                            tsu = sb.tile([64, 512], F8, tag="cv_tsu", bufs=3)
                            nc.scalar.activation(out=tsu[:, :w], in_=tsp[:, :w],
                                                 func=AF.Identity, bias=pb1col[:])
                            d = transpose_scatter(tsu[:, :w], gt, ag_out, col0)
                        scatters[reg].append(d)
                        if hooks and (reg, g) in hooks:
                            hooks[(reg, g)](scatters)
                return scatters

            ccs = {}

            def h_cc3c(sc):
                ccs["3c"] = allgather(ag3_in, PU, SHARD, ts_t, PROD_BASE, sc3c)

            def h_cc2a(sc):
                ccs["2a"] = allgather(ag2_in, 0, P0, y2_t, 0, sc[0])

            cv1 = conv_pass(y1_t, y1ownT, 1, ag2_in, cc1,
                            hooks={(0, 8): h_cc3c, (1, 4): h_cc2a})
            cc2b = allgather(ag2_in, P0, PU, y2_t, UH2, cv1[0] + cv1[1])

            def h_cc3a(sc):
                ccs["3a"] = allgather(ag3_in, 0, P0, ts_t, 0, sc[0])

            cv2 = conv_pass(y2_t, y2ownT, 2, ag3_in, [ccs["2a"], cc2b],
                            hooks={(1, 4): h_cc3a})
            cc3b = allgather(ag3_in, P0, PU, ts_t, UH2, cv2[0] + cv2[1])
            cc3 = [ccs["3a"], cc3b, ccs["3c"]]

            # ================= P7: final pair MLP =================
            offUP_t = cp.tile([128, 2 * NCH], I32, tag="offUP_t")
            doff = nc.sync.dma_start(out=offUP_t[:], in_=offUP_d[:])
            pacc = cp.tile([128, NCH], F32, tag="pacc")
            nneg = 64 - npos
            p7h = {}
            nblk = (NCH // 8 + 3) // 4  # 31 blocks of up to 4 gchunks
            for b in range(nblk):
                g0 = b * 4
                gn = min(4, NCH // 8 - g0)
                ncols16 = gn * 16
                tUP = sb.tile([128, 4096], F8, tag="p7_tUP", bufs=2)
                gt_ = nc.gpsimd.indirect_dma_start(
                    out=tUP[:, :ncols16 * 64], out_offset=None, in_=ts_t[:],
                    in_offset=IndirectOffsetOnAxis(
                        ap=offUP_t[:, 16 * g0:16 * g0 + ncols16], axis=0),
                )
                add_dep_helper(gt_.ins, doff.ins, sync=True, reason="gather reads offsets")
                for _c in cc3:
                    add_dep_helper(gt_.ins, _c.ins, sync=True, reason="gather after AG3")
                if (b - 2) in p7h:
                    add_dep_helper(gt_.ins, p7h[b - 2].ins, sync=True,
                                   reason="WAR tUP slot reuse")
                h8 = sb.tile([128, 2048], BF16, tag="p7_h8", bufs=2)
                a8 = nc.vector.tensor_tensor(
                    out=AP(h8[:].tensor, h8[:].offset,
                           [list(h8[:].ap[0]), [512, gn], [1, 512]]),
                    in0=AP(tUP[:].tensor, tUP[:].offset,
                           [list(tUP[:].ap[0]), [1024, gn], [1, 512]]),
                    in1=AP(tUP[:].tensor, tUP[:].offset + 512,
                           [list(tUP[:].ap[0]), [1024, gn], [1, 512]]),
                    op=ALU.add,
                )
                add_dep_helper(a8.ins, gt_.ins, sync=True, reason="reads tUP")
                p7h[b] = a8
                hr = sb.tile([128, 2048], BF16, tag="p7_hr", bufs=2)
                nc.scalar.activation(out=hr[:, :gn * 512], in_=h8[:, :gn * 512], func=AF.Relu)
                redp = sb.tile([128, 32], F32, tag="p7_redp", bufs=3)
                redn = sb.tile([128, 32], F32, tag="p7_redn", bufs=3)
                ncr = gn * 8
                if npos > 0:
                    nc.vector.tensor_reduce(
                        out=redp[:, :ncr],
                        in_=AP(hr[:].tensor, hr[:].offset,
                               [list(hr[:].ap[0]), [64, ncr], [1, npos]]),
                        axis=mybir.AxisListType.X, op=ALU.add,
                    )
                else:
                    nc.gpsimd.memset(redp[:, :ncr], 0.0)
                if nneg > 0:
                    nc.vector.tensor_reduce(
                        out=redn[:, :ncr],
                        in_=AP(hr[:].tensor, hr[:].offset + npos,
                               [list(hr[:].ap[0]), [64, ncr], [1, nneg]]),
                        axis=mybir.AxisListType.X, op=ALU.add,
                    )
                else:
                    nc.gpsimd.memset(redn[:, :ncr], 0.0)
                df = sb.tile([128, 32], F32, tag="p7_df", bufs=3)
                nc.vector.tensor_tensor(out=df[:, :ncr], in0=redp[:, :ncr],
                                        in1=redn[:, :ncr], op=ALU.subtract)
                sg = sb.tile([128, 32], F32, tag="p7_sg", bufs=3)
                nc.scalar.activation(out=sg[:, :ncr], in_=df[:, :ncr],
                                     func=AF.Sigmoid, bias=b2pred[:], scale=float(inv_g))
                nc.vector.tensor_scalar_mul(out=pacc[:, g0 * 8:g0 * 8 + ncr],
                                            in0=sg[:, :ncr], scalar1=5.0)
            nc.sync.dma_start(out=preds_d[:], in_=pacc[:])

    _split_sync_waits(nc)
    return nc


# --------------------------------------------------------------------------
# runner
# --------------------------------------------------------------------------
def _run(inputs, trace=False):
    per_core, shared, nchA, nchB, npos, inv_g = _prepare(inputs)
    nc = build_program(nchA, nchB, npos, inv_g)
    in_maps = []
    for c in range(N_CORES):
        m = dict(shared)
        m.update(per_core[c])
        in_maps.append({k: np.ascontiguousarray(v) for k, v in m.items()})
    res = run_bass_kernel_spmd(nc, in_maps, core_ids=list(range(N_CORES)), trace=trace)
    out = np.zeros(NE, np.float32)
    el = np.arange(EPT)
    for c in range(N_CORES):
        pc = res.results[c]["preds"]
        out[c * EPT + el] = pc[el % 128, el // 128]
    return out, res.exec_time_ns


def kernel(**inputs):
    out, _ = _run(inputs, trace=False)
    return out
